# revision 21
# baseline (speedup 1.0000x reference)
"""Mixture-of-Experts (B=4, S=2048, D=1024, F=4096, E=8, top-2) on 8 trn2 NeuronCores.

Strategy: load-balanced expert parallelism. The tensor engine is the bottleneck
(bf16 roofline), so per-core work must be leveled. Each expert is split into
2 F-halves x 2 token-halves = 32 units of ~(C_e/2 tokens, F/2 cols). Units are
sorted by token count and packed into 8 cores x 4 fixed-length slots; slot
lengths (compile-time constants, shared by all cores) are the per-group maxima,
so per-core capacity ~= total/8 instead of the max-loaded expert.
  - Host: gate (softmax + top-2 + renorm) in float64, split + gather tokens per
    unit, transpose to [D, L] so the contraction dim lands on SBUF partitions.
  - Device (SPMD, identical program, per-core data): for each of 4 slots s:
    y^T = W2s^T @ gelu(W1s^T @ x^T + b1s) + b2s, weights streamed per slot
    (double buffered), tokens in chunks of <=512, PSUM accumulates in fp32.
    b2 is passed as zeros for F-half-1 units so it is added exactly once.
  - Host: combine with gate weights and scatter-add; the two F-half partials of
    a token-piece add up to the full FFN output.
"""

import copy
import sys

import numpy as np

for _p in ("/opt/trn_rl_repo", "/opt/pypackages"):
    if _p not in sys.path:
        sys.path.append(_p)

import ml_dtypes

B, S, D = 4, 2048, 1024
F = 4 * D
F2 = F // 2
E = 8
TOP_K = 2
P = 128
# PE cost model (HW-measured): a matmul takes N cycles at 2.4 GHz (N = moving
# free dim) + ~2.5 ns NX issue overhead, LDWEIGHTS hidden for N >= 128. So
# big chunks amortize overhead; 512 keeps one PSUM bank per tile.
C_CHUNK = 512
PSUM_W = 512
NSLOT = 4

KO = D // P           # 8  k-subtiles for the first matmul
FT = F2 // P          # 16 f-tiles (partition tiles of h) per F-half
DT = D // P           # 8  d-tiles (partition tiles of y)

# test-harness hooks (left off for grading)
TRACE = False
LAST_RESULTS = None

_compiled = {}


def _split_drain_waits(nc, max_waits=1):
    """This walrus build rejects instructions carrying more than one sync
    wait ("Too many sync wait commands"). Keep one wait on the instruction and
    move the excess onto NoOps inserted right before it on the same engine
    (engines are in-order, so blocking semantics are identical). Updates stay
    on the original instruction — moving them to a trailing NoOp could signal
    before the op's writes land."""
    import concourse.mybir as mybir

    m = nc.m
    new_module = copy.replace(m, functions=[])
    for function in m.functions:
        new_function = copy.replace(function, blocks=[])
        new_function.set_allocations_from_list(function.allocations)
        for block in function.blocks:
            out = []
            for inst in block.instructions:
                si = getattr(inst, "sync_info", None)
                on_wait = list(si.on_wait) if si is not None and si.on_wait else []
                if len(on_wait) > max_waits:
                    engine = getattr(inst, "engine", None)
                    extra, keep = on_wait[max_waits:], on_wait[:max_waits]
                    for j, w in enumerate(extra):
                        out.append(
                            mybir.InstNoOp(
                                name=f"{inst.name}-w{j}",
                                engine=engine,
                                sync_info=mybir.SyncInfo(on_wait=[w], on_update=[]),
                                bass_nofuse=True,
                            )
                        )
                    inst.sync_info = mybir.SyncInfo(
                        on_wait=keep,
                        on_update=list(si.on_update) if si.on_update else [],
                    )
                out.append(inst)
            new_function.blocks.append(copy.replace(block, instructions=out))
        new_module.functions.append(new_function)
    nc.m = new_module
    return nc


def _chunks_of(L):
    """Split L tokens into ceil(L/C_CHUNK) nearly-even chunks (each <= 512)."""
    if L <= 0:
        return []
    n = -(-L // C_CHUNK)
    base, rem = divmod(L, n)
    out, c0 = [], 0
    for i in range(n):
        cn = base + (1 if i < rem else 0)
        out.append((c0, cn))
        c0 += cn
    return out


def _build_nc(Ls):
    """Ls: tuple of NSLOT slot lengths (tokens per slot, same on every core)."""
    import concourse.bass as bass
    import concourse.mybir as mybir
    from concourse.tile import TileContext

    fp32 = mybir.dt.float32
    bf16 = mybir.dt.bfloat16
    AF = mybir.ActivationFunctionType

    T = sum(Ls)
    offs = [sum(Ls[:s]) for s in range(NSLOT)]

    nc = bass.Bass()
    xT = nc.declare_dram_parameter("xT", [D, T], bf16, isOutput=False)
    w1 = nc.declare_dram_parameter("w1", [NSLOT * D, F2], bf16, isOutput=False)
    w2 = nc.declare_dram_parameter("w2", [NSLOT * F2, D], bf16, isOutput=False)
    # biases come host-transposed to [128, n/128] so the DMA is contiguous
    b1 = nc.declare_dram_parameter("b1", [P, NSLOT * FT], fp32, isOutput=False)
    b2 = nc.declare_dram_parameter("b2", [P, NSLOT * DT], fp32, isOutput=False)
    yT = nc.declare_dram_parameter("yT", [D, T], fp32, isOutput=True)

    xT_r = xT.rearrange("(ko ki) t -> ki ko t", ki=P)          # [128, 8, T]
    w1_r = w1.rearrange("(g ki) f -> ki g f", ki=P)            # [128, 4*8, F2]
    w2_r = w2.rearrange("(g fi) d -> fi g d", fi=P)            # [128, 4*16, D]
    yT_r = yT.rearrange("(do di) t -> di do t", di=P)

    seg_chunks = [_chunks_of(L) for L in Ls]

    F_BLK = 512
    FB = F2 // F_BLK   # 4 w1 waves per slot

    with TileContext(nc) as tc:
        with (
            tc.tile_pool(name="wpool", bufs=2) as wpool,
            tc.tile_pool(name="bpool", bufs=1) as bpool,
            # bufs=5: every x DMA's buffer was last read >=1 segment ago, so
            # x DMAs carry no unresolved waits and never head-block the
            # in-order sync DMA queue (the v2 convoy that starved the PE).
            tc.tile_pool(name="xpool", bufs=5) as xpool,
            tc.tile_pool(name="hpool", bufs=1) as hpool,
            tc.tile_pool(name="ypool", bufs=8) as ypool,
            tc.tile_pool(name="hpsum", bufs=4, space="PSUM") as hpsum,
            tc.tile_pool(name="ypsum", bufs=4, space="PSUM") as ypsum,
        ):
            # Weights land in few, large DMAs (6 instructions per slot instead
            # of 48): DMA *issue* costs ~0.65us each on the in-order queue, so
            # many small transfers serialize the queue and delay startup.
            # w1: one [128, KO, 512] tile per f-block; w2: two [128, 8, D].
            w1_t = {}
            w2_t = {}

            def load_weights(s):
                # f-block-major w1 so chunk-0's mm1 can start after block 0;
                # w2 tiles follow (mm2 needs them one chunk later).
                for fb in range(FB):
                    t = wpool.tile([P, KO, F_BLK], bf16, tag=f"w1_{fb}")
                    nc.sync.dma_start(
                        t[:],
                        w1_r[:, s * KO:(s + 1) * KO, fb * F_BLK:(fb + 1) * F_BLK],
                    )
                    w1_t[fb] = t
                for h in range(2):
                    t = wpool.tile([P, FT // 2, D], bf16, tag=f"w2_{h}")
                    nc.sync.dma_start(
                        t[:],
                        w2_r[:, s * FT + h * (FT // 2):s * FT + (h + 1) * (FT // 2), :],
                    )
                    w2_t[h] = t

            def load_x(s, ci):
                c0, cn = seg_chunks[s][ci]
                t = xpool.tile([P, KO, C_CHUNK], bf16, tag="x")
                nc.sync.dma_start(
                    t[:, :, :cn], xT_r[:, :, offs[s] + c0:offs[s] + c0 + cn]
                )
                return t

            # prologue: first x chunk, biases, slot-0 weights, slot-0 x tail,
            # then slot-1 weights + first x so every DMA lands one segment
            # ahead of its consumer.
            # HAM warm-up: the PE clock gate sits at 1.2 GHz until ~3.4us of
            # sustained activity. The PE is idle during the startup DMAs
            # anyway, so burn that window on throwaway matmuls over a zeroed
            # scratch tile; real matmuls then start at 2.4 GHz.
            warm_sb = bpool.tile([P, C_CHUNK], bf16)
            nc.vector.memset(warm_sb[:], 0)
            for _ in range(16):
                w_ps = hpsum.tile([P, PSUM_W], fp32, tag="hps")
                nc.tensor.matmul(
                    w_ps[:, :C_CHUNK], warm_sb[:, :P], warm_sb[:],
                    start=True, stop=True,
                )

            # x00 + biases ride the Activation hwdge queue so the startup load
            # (first x chunk + first w1 block, ~2MB) runs on two queues in
            # parallel; gelu shares that queue but none of these carry waits.
            x_tiles = {}
            t = xpool.tile([P, KO, C_CHUNK], bf16, tag="x")
            c0_0, cn_0 = seg_chunks[0][0]
            nc.scalar.dma_start(t[:, :, :cn_0], xT_r[:, :, c0_0:c0_0 + cn_0])
            x_tiles[(0, 0)] = t
            b1_sb = bpool.tile([P, NSLOT * FT], fp32)
            nc.scalar.dma_start(b1_sb[:], b1[:])
            b2_sb = bpool.tile([P, NSLOT * DT], fp32)
            nc.scalar.dma_start(b2_sb[:], b2[:])
            load_weights(0)
            cur_w1 = dict(w1_t)
            cur_w2 = dict(w2_t)
            for ci in range(1, len(seg_chunks[0])):
                x_tiles[(0, ci)] = load_x(0, ci)
            if NSLOT > 1:
                load_weights(1)
                nxt_w1, nxt_w2 = dict(w1_t), dict(w2_t)
                x_tiles[(1, 0)] = load_x(1, 0)

            for s in range(NSLOT):
                if s > 0:
                    cur_w1, cur_w2 = nxt_w1, nxt_w2
                    # queue the rest of this segment's x, then next segment's
                    # weights and first x chunk.
                    for ci in range(1, len(seg_chunks[s])):
                        x_tiles[(s, ci)] = load_x(s, ci)
                    if s + 1 < NSLOT:
                        load_weights(s + 1)
                        nxt_w1, nxt_w2 = dict(w1_t), dict(w2_t)
                        x_tiles[(s + 1, 0)] = load_x(s + 1, 0)

                for ci, (c0, cn) in enumerate(seg_chunks[s]):
                    x_sb = x_tiles.pop((s, ci))

                    h_sb = hpool.tile([P, FT, C_CHUNK], bf16, tag="h")
                    for ft in range(FT):
                        fb, fc = divmod(ft * P, F_BLK)
                        h_ps = hpsum.tile([P, PSUM_W], fp32, tag="hps")
                        for ko in range(KO):
                            nc.tensor.matmul(
                                h_ps[:, :cn],
                                cur_w1[fb][:, ko, fc:fc + P],
                                x_sb[:, ko, :cn],
                                start=(ko == 0),
                                stop=(ko == KO - 1),
                            )
                        # gelu(mm + b1) fused on ScalarE, cast to bf16 on write
                        nc.scalar.activation(
                            h_sb[:, ft, :cn], h_ps[:, :cn], AF.Gelu,
                            bias=b1_sb[:, s * FT + ft:s * FT + ft + 1],
                        )

                    last_chunk = (s == NSLOT - 1) and (ci == len(seg_chunks[s]) - 1)
                    for dt_ in range(DT):
                        y_ps = ypsum.tile([P, PSUM_W], fp32, tag="yps")
                        for fo in range(FT):
                            nc.tensor.matmul(
                                y_ps[:, :cn],
                                cur_w2[fo // 8][:, fo % 8, dt_ * P:(dt_ + 1) * P],
                                h_sb[:, fo, :cn],
                                start=(fo == 0),
                                stop=(fo == FT - 1),
                            )
                        y_sb = ypool.tile([P, C_CHUNK], fp32, tag="y")
                        nc.vector.tensor_scalar_add(
                            y_sb[:, :cn], y_ps[:, :cn],
                            b2_sb[:, s * DT + dt_:s * DT + dt_ + 1],
                        )
                        # y-out stays on the sync queue: by the time a y DMA
                        # reaches the queue head its vector add has already
                        # run, and nothing urgent queues behind it. (On the
                        # Activation queue it would block the next chunk's
                        # gelu stream and stall the PE via PSUM backpressure.)
                        # The final chunk splits across both queues to halve
                        # the drain at kernel end (gelu is done by then).
                        y_eng = nc.scalar if (last_chunk and dt_ % 2) else nc.sync
                        y_eng.dma_start(
                            yT_r[:, dt_, offs[s] + c0:offs[s] + c0 + cn],
                            y_sb[:, :cn],
                        )

    return _split_drain_waits(nc)


def _to_bf16(a):
    """Fast float32 -> bfloat16 with round-to-nearest-even via bit ops."""
    a = np.ascontiguousarray(a, dtype=np.float32)
    u = a.view(np.uint32)
    r = ((u + 0x7FFF + ((u >> 16) & 1)) >> 16).astype(np.uint16)
    return r.view(ml_dtypes.bfloat16)


def kernel(hidden_states, Wg, bg, W1, b1, W2, b2):
    from concourse import bass_utils

    hs = np.ascontiguousarray(hidden_states, dtype=np.float32).reshape(B * S, D)

    # ---- Gate on host (float64): softmax over experts, top-2, renormalize
    logits = hs.astype(np.float64) @ np.asarray(Wg, np.float64).T
    logits += np.asarray(bg, np.float64)
    logits -= logits.max(axis=-1, keepdims=True)
    p = np.exp(logits)
    p /= p.sum(axis=-1, keepdims=True)

    i1 = p.argmax(axis=-1)
    rows = np.arange(B * S)
    p1 = p[rows, i1]
    pm = p.copy()
    pm[rows, i1] = -1.0
    i2 = pm.argmax(axis=-1)
    p2 = p[rows, i2]
    denom = p1 + p2
    g1 = (p1 / denom).astype(np.float32)
    g2 = (p2 / denom).astype(np.float32)

    # ---- Dispatch: token ids + combine weights per expert, split into two
    # token-halves; each half is served by two units (one per F-half).
    pieces = []  # (length, expert, ids, cw)
    for e in range(E):
        m1 = np.nonzero(i1 == e)[0]
        m2 = np.nonzero(i2 == e)[0]
        ids = np.concatenate([m1, m2])
        cw = np.concatenate([g1[m1], g2[m2]])
        # Even halves minimize total padded capacity (sum of group maxima).
        n0 = (len(ids) + 1) // 2
        for sl in (slice(0, n0), slice(n0, len(ids))):
            pieces.append((len(ids[sl]), e, ids[sl], cw[sl]))

    # 32 units = 16 token-pieces x 2 F-halves; sort by length, pack into
    # NSLOT groups of 8; slot length = group max; unit i of group g goes to
    # core i, slot g.
    units = []  # (length, piece_idx, fhalf)
    for pi, (ln, _e, _ids, _cw) in enumerate(pieces):
        for fh in range(2):
            units.append((ln, pi, fh))
    units.sort(key=lambda u: -u[0])
    assert len(units) == 8 * NSLOT
    groups = [units[8 * g:8 * (g + 1)] for g in range(NSLOT)]
    Ls = tuple(max(max(u[0] for u in grp), 1) for grp in groups)
    offs = [sum(Ls[:s]) for s in range(NSLOT)]
    T = sum(Ls)

    if Ls not in _compiled:
        _compiled[Ls] = _build_nc(Ls)
    nc = _compiled[Ls]

    W1b = [_to_bf16(np.asarray(W1[e], np.float32)) for e in range(E)]
    W2b = [_to_bf16(np.asarray(W2[e], np.float32)) for e in range(E)]

    in_maps = []
    placement = []  # per core: list over slots of (piece_idx, fhalf)
    for c in range(E):
        xT = np.zeros((D, T), dtype=ml_dtypes.bfloat16)
        w1s = np.zeros((NSLOT * D, F2), dtype=ml_dtypes.bfloat16)
        w2s = np.zeros((NSLOT * F2, D), dtype=ml_dtypes.bfloat16)
        b1s = np.zeros((P, NSLOT * FT), dtype=np.float32)
        b2s = np.zeros((P, NSLOT * DT), dtype=np.float32)
        slots = []
        for g in range(NSLOT):
            ln, pi, fh = groups[g][c]
            _ln, e, ids, _cw = pieces[pi]
            fsl = slice(fh * F2, (fh + 1) * F2)
            xT[:, offs[g]:offs[g] + ln] = _to_bf16(hs[ids]).T
            w1s[g * D:(g + 1) * D] = W1b[e][:, fsl]
            w2s[g * F2:(g + 1) * F2] = W2b[e][fsl]
            b1s[:, g * FT:(g + 1) * FT] = (
                np.asarray(b1[e], np.float32)[fsl].reshape(FT, P).T
            )
            if fh == 0:  # add b2 exactly once per token-piece
                b2s[:, g * DT:(g + 1) * DT] = (
                    np.asarray(b2[e], np.float32).reshape(DT, P).T
                )
            slots.append((pi, fh))
        placement.append(slots)
        in_maps.append({
            "xT": xT, "w1": w1s, "w2": w2s, "b1": b1s, "b2": b2s,
        })

    kwargs = {}
    if TRACE:
        import os as _os
        kwargs = dict(trace=True, trace_cores=list(range(E)))
        if _os.environ.get("MOE_TRACE_DIR"):
            _os.makedirs(_os.environ["MOE_TRACE_DIR"], exist_ok=True)
            kwargs["tmpdir"] = _os.environ["MOE_TRACE_DIR"]
    res = bass_utils.run_bass_kernel_spmd(nc, in_maps, list(range(E)), **kwargs)
    global LAST_RESULTS
    LAST_RESULTS = res

    out = np.zeros((B * S, D), dtype=np.float32)
    for c in range(E):
        yTc = res.results[c]["yT"]
        for g in range(NSLOT):
            pi, _fh = placement[c][g]
            ln, _e, ids, cw = pieces[pi]
            if ln:
                out[ids] += cw[:, None] * yTc[:, offs[g]:offs[g] + ln].T
    return out.reshape(B, S, D)


# revision 25
# speedup vs baseline: 1.1685x; 1.1685x over previous
"""Mixture-of-Experts (B=4, S=2048, D=1024, F=4096, E=8, top-2) on 8 trn2 NeuronCores.

Strategy: load-balanced expert parallelism. The tensor engine is the bottleneck
(bf16 roofline), so per-core work must be leveled. Each expert is split into
2 F-halves x 2 token-halves = 32 units of ~(C_e/2 tokens, F/2 cols). Units are
sorted by token count and packed into 8 cores x 4 fixed-length slots; slot
lengths (compile-time constants, shared by all cores) are the per-group maxima,
so per-core capacity ~= total/8 instead of the max-loaded expert.
  - Host: gate (softmax + top-2 + renorm) in float64, split + gather tokens per
    unit, transpose to [D, L] so the contraction dim lands on SBUF partitions.
  - Device (SPMD, identical program, per-core data): for each of 4 slots s:
    y^T = W2s^T @ gelu(W1s^T @ x^T + b1s) + b2s, weights streamed per slot
    (double buffered), tokens in chunks of <=512, PSUM accumulates in fp32.
    b2 is passed as zeros for F-half-1 units so it is added exactly once.
  - Host: combine with gate weights and scatter-add; the two F-half partials of
    a token-piece add up to the full FFN output.
"""

import copy
import sys

import numpy as np

for _p in ("/opt/trn_rl_repo", "/opt/pypackages"):
    if _p not in sys.path:
        sys.path.append(_p)

import ml_dtypes

B, S, D = 4, 2048, 1024
F = 4 * D
F2 = F // 2
E = 8
TOP_K = 2
P = 128
# PE cost model (HW-measured): a matmul takes N cycles at 2.4 GHz (N = moving
# free dim) + ~2.5 ns NX issue overhead, LDWEIGHTS hidden for N >= 128. So
# big chunks amortize overhead; 512 keeps one PSUM bank per tile.
C_CHUNK = 512
PSUM_W = 512
NSLOT = 4

KO = D // P           # 8  k-subtiles for the first matmul
FT = F2 // P          # 16 f-tiles (partition tiles of h) per F-half
DT = D // P           # 8  d-tiles (partition tiles of y)

# test-harness hooks (left off for grading)
TRACE = False
LAST_RESULTS = None

_compiled = {}


def _split_drain_waits(nc, max_waits=1):
    """This walrus build rejects instructions carrying more than one sync
    wait ("Too many sync wait commands"). Keep one wait on the instruction and
    move the excess onto NoOps inserted right before it on the same engine
    (engines are in-order, so blocking semantics are identical). Updates stay
    on the original instruction — moving them to a trailing NoOp could signal
    before the op's writes land."""
    import concourse.mybir as mybir

    m = nc.m
    new_module = copy.replace(m, functions=[])
    for function in m.functions:
        new_function = copy.replace(function, blocks=[])
        new_function.set_allocations_from_list(function.allocations)
        for block in function.blocks:
            out = []
            for inst in block.instructions:
                si = getattr(inst, "sync_info", None)
                on_wait = list(si.on_wait) if si is not None and si.on_wait else []
                if len(on_wait) > max_waits:
                    engine = getattr(inst, "engine", None)
                    extra, keep = on_wait[max_waits:], on_wait[:max_waits]
                    for j, w in enumerate(extra):
                        out.append(
                            mybir.InstNoOp(
                                name=f"{inst.name}-w{j}",
                                engine=engine,
                                sync_info=mybir.SyncInfo(on_wait=[w], on_update=[]),
                                bass_nofuse=True,
                            )
                        )
                    inst.sync_info = mybir.SyncInfo(
                        on_wait=keep,
                        on_update=list(si.on_update) if si.on_update else [],
                    )
                out.append(inst)
            new_function.blocks.append(copy.replace(block, instructions=out))
        new_module.functions.append(new_function)
    nc.m = new_module
    return nc


def _chunks_of(L):
    """Split L tokens into ceil(L/C_CHUNK) nearly-even chunks (each <= 512)."""
    if L <= 0:
        return []
    n = -(-L // C_CHUNK)
    base, rem = divmod(L, n)
    out, c0 = [], 0
    for i in range(n):
        cn = base + (1 if i < rem else 0)
        out.append((c0, cn))
        c0 += cn
    return out


def _build_nc(Ls):
    """Ls: tuple of NSLOT slot lengths (tokens per slot, same on every core)."""
    import concourse.bass as bass
    import concourse.mybir as mybir
    from concourse.tile import TileContext

    fp32 = mybir.dt.float32
    bf16 = mybir.dt.bfloat16
    AF = mybir.ActivationFunctionType

    T = sum(Ls)
    offs = [sum(Ls[:s]) for s in range(NSLOT)]

    nc = bass.Bass()
    xT = nc.declare_dram_parameter("xT", [D, T], bf16, isOutput=False)
    w1 = nc.declare_dram_parameter("w1", [NSLOT * D, F2], bf16, isOutput=False)
    w2 = nc.declare_dram_parameter("w2", [NSLOT * F2, D], bf16, isOutput=False)
    # biases come host-transposed to [128, n/128] so the DMA is contiguous
    b1 = nc.declare_dram_parameter("b1", [P, NSLOT * FT], fp32, isOutput=False)
    b2 = nc.declare_dram_parameter("b2", [P, NSLOT * DT], fp32, isOutput=False)
    yT = nc.declare_dram_parameter("yT", [D, T], fp32, isOutput=True)

    xT_r = xT.rearrange("(ko ki) t -> ki ko t", ki=P)          # [128, 8, T]
    w1_r = w1.rearrange("(g ki) f -> ki g f", ki=P)            # [128, 4*8, F2]
    w2_r = w2.rearrange("(g fi) d -> fi g d", fi=P)            # [128, 4*16, D]
    yT_r = yT.rearrange("(do di) t -> di do t", di=P)

    seg_chunks = [_chunks_of(L) for L in Ls]

    F_BLK = 512
    FB = F2 // F_BLK   # 4 w1 waves per slot

    with TileContext(nc) as tc:
        with (
            tc.tile_pool(name="wpool", bufs=2) as wpool,
            tc.tile_pool(name="bpool", bufs=1) as bpool,
            # bufs=5: every x DMA's buffer was last read >=1 segment ago, so
            # x DMAs carry no unresolved waits and never head-block the
            # in-order sync DMA queue (the v2 convoy that starved the PE).
            tc.tile_pool(name="xpool", bufs=5) as xpool,
            tc.tile_pool(name="hpool", bufs=1) as hpool,
            tc.tile_pool(name="ypool", bufs=8) as ypool,
            tc.tile_pool(name="hpsum", bufs=4, space="PSUM") as hpsum,
            tc.tile_pool(name="ypsum", bufs=4, space="PSUM") as ypsum,
        ):
            # Weight tiles stay flat [128, 512] / [128, 1024]: LDWEIGHTS from
            # slices of batched 3D tiles measures ~20 ns slower PER MATMUL
            # (+50us/kernel — it defeats the fast-weight-load path). The many
            # small DMAs' issue cost overlaps compute and is harmless.
            w1_t = {}
            w2_t = {}

            def load_weights(s):
                # f-block-major w1 waves so chunk-0's mm1 can start after
                # wave 0; w2 tiles follow (mm2 needs them one chunk later).
                for fb in range(FB):
                    for ko in range(KO):
                        t = wpool.tile([P, F_BLK], bf16, tag=f"w1_{ko}_{fb}")
                        nc.sync.dma_start(
                            t[:], w1_r[:, s * KO + ko, fb * F_BLK:(fb + 1) * F_BLK]
                        )
                        w1_t[(ko, fb)] = t
                for fo in range(FT):
                    t = wpool.tile([P, D], bf16, tag=f"w2_{fo}")
                    nc.sync.dma_start(t[:], w2_r[:, s * FT + fo, :])
                    w2_t[fo] = t

            def load_x(s, ci):
                c0, cn = seg_chunks[s][ci]
                t = xpool.tile([P, KO, C_CHUNK], bf16, tag="x")
                nc.sync.dma_start(
                    t[:, :, :cn], xT_r[:, :, offs[s] + c0:offs[s] + c0 + cn]
                )
                return t

            # prologue: first x chunk, biases, slot-0 weights, slot-0 x tail,
            # then slot-1 weights + first x so every DMA lands one segment
            # ahead of its consumer.
            # HAM warm-up: the PE clock gate sits at 1.2 GHz until ~3.4us of
            # sustained activity. The PE is idle during the startup DMAs
            # anyway, so burn that window on throwaway matmuls over a zeroed
            # scratch tile; real matmuls then start at 2.4 GHz.
            warm_sb = bpool.tile([P, C_CHUNK], bf16)
            nc.vector.memset(warm_sb[:], 0)
            for _ in range(36):
                w_ps = hpsum.tile([P, PSUM_W], fp32, tag="hps")
                nc.tensor.matmul(
                    w_ps[:, :C_CHUNK], warm_sb[:, :P], warm_sb[:],
                    start=True, stop=True,
                )

            # x00 + biases ride the Activation hwdge queue so the startup load
            # (first x chunk + first w1 block, ~2MB) runs on two queues in
            # parallel; gelu shares that queue but none of these carry waits.
            x_tiles = {}
            t = xpool.tile([P, KO, C_CHUNK], bf16, tag="x")
            c0_0, cn_0 = seg_chunks[0][0]
            nc.scalar.dma_start(t[:, :, :cn_0], xT_r[:, :, c0_0:c0_0 + cn_0])
            x_tiles[(0, 0)] = t
            b1_sb = bpool.tile([P, NSLOT * FT], fp32)
            nc.scalar.dma_start(b1_sb[:], b1[:])
            b2_sb = bpool.tile([P, NSLOT * DT], fp32)
            nc.scalar.dma_start(b2_sb[:], b2[:])
            load_weights(0)
            cur_w1 = dict(w1_t)
            cur_w2 = dict(w2_t)
            for ci in range(1, len(seg_chunks[0])):
                x_tiles[(0, ci)] = load_x(0, ci)
            if NSLOT > 1:
                load_weights(1)
                nxt_w1, nxt_w2 = dict(w1_t), dict(w2_t)
                x_tiles[(1, 0)] = load_x(1, 0)

            for s in range(NSLOT):
                if s > 0:
                    cur_w1, cur_w2 = nxt_w1, nxt_w2
                    # queue the rest of this segment's x, then next segment's
                    # weights and first x chunk.
                    for ci in range(1, len(seg_chunks[s])):
                        x_tiles[(s, ci)] = load_x(s, ci)
                    if s + 1 < NSLOT:
                        load_weights(s + 1)
                        nxt_w1, nxt_w2 = dict(w1_t), dict(w2_t)
                        x_tiles[(s + 1, 0)] = load_x(s + 1, 0)

                for ci, (c0, cn) in enumerate(seg_chunks[s]):
                    x_sb = x_tiles.pop((s, ci))

                    h_sb = hpool.tile([P, FT, C_CHUNK], bf16, tag="h")
                    for ft in range(FT):
                        fb, fc = divmod(ft * P, F_BLK)
                        h_ps = hpsum.tile([P, PSUM_W], fp32, tag="hps")
                        for ko in range(KO):
                            nc.tensor.matmul(
                                h_ps[:, :cn],
                                cur_w1[(ko, fb)][:, fc:fc + P],
                                x_sb[:, ko, :cn],
                                start=(ko == 0),
                                stop=(ko == KO - 1),
                            )
                        # gelu(mm + b1) fused on ScalarE, cast to bf16 on write
                        nc.scalar.activation(
                            h_sb[:, ft, :cn], h_ps[:, :cn], AF.Gelu,
                            bias=b1_sb[:, s * FT + ft:s * FT + ft + 1],
                        )

                    last_chunk = (s == NSLOT - 1) and (ci == len(seg_chunks[s]) - 1)
                    for dt_ in range(DT):
                        y_ps = ypsum.tile([P, PSUM_W], fp32, tag="yps")
                        for fo in range(FT):
                            nc.tensor.matmul(
                                y_ps[:, :cn],
                                cur_w2[fo][:, dt_ * P:(dt_ + 1) * P],
                                h_sb[:, fo, :cn],
                                start=(fo == 0),
                                stop=(fo == FT - 1),
                            )
                        y_sb = ypool.tile([P, C_CHUNK], fp32, tag="y")
                        nc.vector.tensor_scalar_add(
                            y_sb[:, :cn], y_ps[:, :cn],
                            b2_sb[:, s * DT + dt_:s * DT + dt_ + 1],
                        )
                        # y-out stays on the sync queue: by the time a y DMA
                        # reaches the queue head its vector add has already
                        # run, and nothing urgent queues behind it. (On the
                        # Activation queue it would block the next chunk's
                        # gelu stream and stall the PE via PSUM backpressure.)
                        # The final chunk splits across both queues to halve
                        # the drain at kernel end (gelu is done by then).
                        y_eng = nc.scalar if (last_chunk and dt_ % 2) else nc.sync
                        y_eng.dma_start(
                            yT_r[:, dt_, offs[s] + c0:offs[s] + c0 + cn],
                            y_sb[:, :cn],
                        )

    return _split_drain_waits(nc)


def _to_bf16(a):
    """Fast float32 -> bfloat16 with round-to-nearest-even via bit ops."""
    a = np.ascontiguousarray(a, dtype=np.float32)
    u = a.view(np.uint32)
    r = ((u + 0x7FFF + ((u >> 16) & 1)) >> 16).astype(np.uint16)
    return r.view(ml_dtypes.bfloat16)


def kernel(hidden_states, Wg, bg, W1, b1, W2, b2):
    from concourse import bass_utils

    hs = np.ascontiguousarray(hidden_states, dtype=np.float32).reshape(B * S, D)

    # ---- Gate on host (float64): softmax over experts, top-2, renormalize
    logits = hs.astype(np.float64) @ np.asarray(Wg, np.float64).T
    logits += np.asarray(bg, np.float64)
    logits -= logits.max(axis=-1, keepdims=True)
    p = np.exp(logits)
    p /= p.sum(axis=-1, keepdims=True)

    i1 = p.argmax(axis=-1)
    rows = np.arange(B * S)
    p1 = p[rows, i1]
    pm = p.copy()
    pm[rows, i1] = -1.0
    i2 = pm.argmax(axis=-1)
    p2 = p[rows, i2]
    denom = p1 + p2
    g1 = (p1 / denom).astype(np.float32)
    g2 = (p2 / denom).astype(np.float32)

    # ---- Dispatch: token ids + combine weights per expert, split into two
    # token-halves; each half is served by two units (one per F-half).
    pieces = []  # (length, expert, ids, cw)
    for e in range(E):
        m1 = np.nonzero(i1 == e)[0]
        m2 = np.nonzero(i2 == e)[0]
        ids = np.concatenate([m1, m2])
        cw = np.concatenate([g1[m1], g2[m2]])
        # Even halves minimize total padded capacity (sum of group maxima).
        n0 = (len(ids) + 1) // 2
        for sl in (slice(0, n0), slice(n0, len(ids))):
            pieces.append((len(ids[sl]), e, ids[sl], cw[sl]))

    # 32 units = 16 token-pieces x 2 F-halves; sort by length, pack into
    # NSLOT groups of 8; slot length = group max; unit i of group g goes to
    # core i, slot g.
    units = []  # (length, piece_idx, fhalf)
    for pi, (ln, _e, _ids, _cw) in enumerate(pieces):
        for fh in range(2):
            units.append((ln, pi, fh))
    units.sort(key=lambda u: -u[0])
    assert len(units) == 8 * NSLOT
    groups = [units[8 * g:8 * (g + 1)] for g in range(NSLOT)]
    Ls = tuple(max(max(u[0] for u in grp), 1) for grp in groups)
    offs = [sum(Ls[:s]) for s in range(NSLOT)]
    T = sum(Ls)

    if Ls not in _compiled:
        _compiled[Ls] = _build_nc(Ls)
    nc = _compiled[Ls]

    W1b = [_to_bf16(np.asarray(W1[e], np.float32)) for e in range(E)]
    W2b = [_to_bf16(np.asarray(W2[e], np.float32)) for e in range(E)]

    in_maps = []
    placement = []  # per core: list over slots of (piece_idx, fhalf)
    for c in range(E):
        xT = np.zeros((D, T), dtype=ml_dtypes.bfloat16)
        w1s = np.zeros((NSLOT * D, F2), dtype=ml_dtypes.bfloat16)
        w2s = np.zeros((NSLOT * F2, D), dtype=ml_dtypes.bfloat16)
        b1s = np.zeros((P, NSLOT * FT), dtype=np.float32)
        b2s = np.zeros((P, NSLOT * DT), dtype=np.float32)
        slots = []
        for g in range(NSLOT):
            ln, pi, fh = groups[g][c]
            _ln, e, ids, _cw = pieces[pi]
            fsl = slice(fh * F2, (fh + 1) * F2)
            xT[:, offs[g]:offs[g] + ln] = _to_bf16(hs[ids]).T
            w1s[g * D:(g + 1) * D] = W1b[e][:, fsl]
            w2s[g * F2:(g + 1) * F2] = W2b[e][fsl]
            b1s[:, g * FT:(g + 1) * FT] = (
                np.asarray(b1[e], np.float32)[fsl].reshape(FT, P).T
            )
            if fh == 0:  # add b2 exactly once per token-piece
                b2s[:, g * DT:(g + 1) * DT] = (
                    np.asarray(b2[e], np.float32).reshape(DT, P).T
                )
            slots.append((pi, fh))
        placement.append(slots)
        in_maps.append({
            "xT": xT, "w1": w1s, "w2": w2s, "b1": b1s, "b2": b2s,
        })

    kwargs = {}
    if TRACE:
        import os as _os
        kwargs = dict(trace=True, trace_cores=list(range(E)))
        if _os.environ.get("MOE_TRACE_DIR"):
            _os.makedirs(_os.environ["MOE_TRACE_DIR"], exist_ok=True)
            kwargs["tmpdir"] = _os.environ["MOE_TRACE_DIR"]
    res = bass_utils.run_bass_kernel_spmd(nc, in_maps, list(range(E)), **kwargs)
    global LAST_RESULTS
    LAST_RESULTS = res

    out = np.zeros((B * S, D), dtype=np.float32)
    for c in range(E):
        yTc = res.results[c]["yT"]
        for g in range(NSLOT):
            pi, _fh = placement[c][g]
            ln, _e, ids, cw = pieces[pi]
            if ln:
                out[ids] += cw[:, None] * yTc[:, offs[g]:offs[g] + ln].T
    return out.reshape(B, S, D)


# revision 26
# speedup vs baseline: 1.1714x; 1.0025x over previous
"""Mixture-of-Experts (B=4, S=2048, D=1024, F=4096, E=8, top-2) on 8 trn2 NeuronCores.

Strategy: load-balanced expert parallelism. The tensor engine is the bottleneck
(bf16 roofline), so per-core work must be leveled. Each expert is split into
2 F-halves x 2 token-halves = 32 units of ~(C_e/2 tokens, F/2 cols). Units are
sorted by token count and packed into 8 cores x 4 fixed-length slots; slot
lengths (compile-time constants, shared by all cores) are the per-group maxima,
so per-core capacity ~= total/8 instead of the max-loaded expert.
  - Host: gate (softmax + top-2 + renorm) in float64, split + gather tokens per
    unit, transpose to [D, L] so the contraction dim lands on SBUF partitions.
  - Device (SPMD, identical program, per-core data): for each of 4 slots s:
    y^T = W2s^T @ gelu(W1s^T @ x^T + b1s) + b2s, weights streamed per slot
    (double buffered), tokens in chunks of <=512, PSUM accumulates in fp32.
    b2 is passed as zeros for F-half-1 units so it is added exactly once.
  - Host: combine with gate weights and scatter-add; the two F-half partials of
    a token-piece add up to the full FFN output.
"""

import copy
import sys

import numpy as np

for _p in ("/opt/trn_rl_repo", "/opt/pypackages"):
    if _p not in sys.path:
        sys.path.append(_p)

import ml_dtypes

B, S, D = 4, 2048, 1024
F = 4 * D
F2 = F // 2
E = 8
TOP_K = 2
P = 128
# PE cost model (HW-measured): a matmul takes N cycles at 2.4 GHz (N = moving
# free dim) + ~2.5 ns NX issue overhead, LDWEIGHTS hidden for N >= 128. So
# big chunks amortize overhead; 512 keeps one PSUM bank per tile.
C_CHUNK = 512
PSUM_W = 512
NSLOT = 4

KO = D // P           # 8  k-subtiles for the first matmul
FT = F2 // P          # 16 f-tiles (partition tiles of h) per F-half
DT = D // P           # 8  d-tiles (partition tiles of y)

# test-harness hooks (left off for grading)
TRACE = False
LAST_RESULTS = None

_compiled = {}


def _split_drain_waits(nc, max_waits=1):
    """This walrus build rejects instructions carrying more than one sync
    wait ("Too many sync wait commands"). Keep one wait on the instruction and
    move the excess onto NoOps inserted right before it on the same engine
    (engines are in-order, so blocking semantics are identical). Updates stay
    on the original instruction — moving them to a trailing NoOp could signal
    before the op's writes land."""
    import concourse.mybir as mybir

    m = nc.m
    new_module = copy.replace(m, functions=[])
    for function in m.functions:
        new_function = copy.replace(function, blocks=[])
        new_function.set_allocations_from_list(function.allocations)
        for block in function.blocks:
            out = []
            for inst in block.instructions:
                si = getattr(inst, "sync_info", None)
                on_wait = list(si.on_wait) if si is not None and si.on_wait else []
                if len(on_wait) > max_waits:
                    engine = getattr(inst, "engine", None)
                    extra, keep = on_wait[max_waits:], on_wait[:max_waits]
                    for j, w in enumerate(extra):
                        out.append(
                            mybir.InstNoOp(
                                name=f"{inst.name}-w{j}",
                                engine=engine,
                                sync_info=mybir.SyncInfo(on_wait=[w], on_update=[]),
                                bass_nofuse=True,
                            )
                        )
                    inst.sync_info = mybir.SyncInfo(
                        on_wait=keep,
                        on_update=list(si.on_update) if si.on_update else [],
                    )
                out.append(inst)
            new_function.blocks.append(copy.replace(block, instructions=out))
        new_module.functions.append(new_function)
    nc.m = new_module
    return nc


def _chunks_of(L):
    """Split L tokens into ceil(L/C_CHUNK) nearly-even chunks (each <= 512)."""
    if L <= 0:
        return []
    n = -(-L // C_CHUNK)
    base, rem = divmod(L, n)
    out, c0 = [], 0
    for i in range(n):
        cn = base + (1 if i < rem else 0)
        out.append((c0, cn))
        c0 += cn
    return out


def _build_nc(Ls):
    """Ls: tuple of NSLOT slot lengths (tokens per slot, same on every core)."""
    import concourse.bass as bass
    import concourse.mybir as mybir
    from concourse.tile import TileContext

    fp32 = mybir.dt.float32
    bf16 = mybir.dt.bfloat16
    AF = mybir.ActivationFunctionType

    T = sum(Ls)
    offs = [sum(Ls[:s]) for s in range(NSLOT)]

    nc = bass.Bass()
    xT = nc.declare_dram_parameter("xT", [D, T], bf16, isOutput=False)
    w1 = nc.declare_dram_parameter("w1", [NSLOT * D, F2], bf16, isOutput=False)
    w2 = nc.declare_dram_parameter("w2", [NSLOT * F2, D], bf16, isOutput=False)
    # biases come host-transposed to [128, n/128] so the DMA is contiguous
    b1 = nc.declare_dram_parameter("b1", [P, NSLOT * FT], fp32, isOutput=False)
    b2 = nc.declare_dram_parameter("b2", [P, NSLOT * DT], fp32, isOutput=False)
    yT = nc.declare_dram_parameter("yT", [D, T], fp32, isOutput=True)

    xT_r = xT.rearrange("(ko ki) t -> ki ko t", ki=P)          # [128, 8, T]
    w1_r = w1.rearrange("(g ki) f -> ki g f", ki=P)            # [128, 4*8, F2]
    w2_r = w2.rearrange("(g fi) d -> fi g d", fi=P)            # [128, 4*16, D]
    yT_r = yT.rearrange("(do di) t -> di do t", di=P)

    seg_chunks = [_chunks_of(L) for L in Ls]

    F_BLK = 512
    FB = F2 // F_BLK   # 4 w1 waves per slot

    with TileContext(nc) as tc:
        with (
            tc.tile_pool(name="wpool", bufs=2) as wpool,
            tc.tile_pool(name="bpool", bufs=1) as bpool,
            # bufs=5: every x DMA's buffer was last read >=1 segment ago, so
            # x DMAs carry no unresolved waits and never head-block the
            # in-order sync DMA queue (the v2 convoy that starved the PE).
            tc.tile_pool(name="xpool", bufs=5) as xpool,
            tc.tile_pool(name="hpool", bufs=1) as hpool,
            tc.tile_pool(name="ypool", bufs=8) as ypool,
            tc.tile_pool(name="hpsum", bufs=4, space="PSUM") as hpsum,
            tc.tile_pool(name="ypsum", bufs=4, space="PSUM") as ypsum,
        ):
            # Weight tiles stay flat [128, 512] / [128, 1024]: LDWEIGHTS from
            # slices of batched 3D tiles measures ~20 ns slower PER MATMUL
            # (+50us/kernel — it defeats the fast-weight-load path). The many
            # small DMAs' issue cost overlaps compute and is harmless.
            w1_t = {}
            w2_t = {}

            def load_weights(s):
                # f-block-major w1 waves so chunk-0's mm1 can start after
                # wave 0; w2 tiles follow (mm2 needs them one chunk later).
                for fb in range(FB):
                    for ko in range(KO):
                        t = wpool.tile([P, F_BLK], bf16, tag=f"w1_{ko}_{fb}")
                        nc.sync.dma_start(
                            t[:], w1_r[:, s * KO + ko, fb * F_BLK:(fb + 1) * F_BLK]
                        )
                        w1_t[(ko, fb)] = t
                for fo in range(FT):
                    t = wpool.tile([P, D], bf16, tag=f"w2_{fo}")
                    nc.sync.dma_start(t[:], w2_r[:, s * FT + fo, :])
                    w2_t[fo] = t

            def load_x(s, ci):
                c0, cn = seg_chunks[s][ci]
                t = xpool.tile([P, KO, C_CHUNK], bf16, tag="x")
                nc.sync.dma_start(
                    t[:, :, :cn], xT_r[:, :, offs[s] + c0:offs[s] + c0 + cn]
                )
                return t

            # prologue: first x chunk, biases, slot-0 weights, slot-0 x tail,
            # then slot-1 weights + first x so every DMA lands one segment
            # ahead of its consumer.
            # HAM warm-up: the PE clock gate sits at 1.2 GHz until ~3.4us of
            # sustained activity. The PE is idle during the startup DMAs
            # anyway, so burn that window on throwaway matmuls over a zeroed
            # scratch tile; real matmuls then start at 2.4 GHz.
            # Engine preamble ends ~7us and the first real matmul's data lands
            # ~13us; 24 N=256 throwaway matmuls fill exactly that window
            # (~3.4us of them cold, which is what flips the clock gate).
            warm_sb = bpool.tile([P, C_CHUNK], bf16)
            nc.vector.memset(warm_sb[:], 0)
            for _ in range(24):
                w_ps = hpsum.tile([P, PSUM_W], fp32, tag="hps")
                nc.tensor.matmul(
                    w_ps[:, :256], warm_sb[:, :P], warm_sb[:, :256],
                    start=True, stop=True,
                )

            # x00 + biases ride the Activation hwdge queue so the startup load
            # (first x chunk + first w1 block, ~2MB) runs on two queues in
            # parallel; gelu shares that queue but none of these carry waits.
            x_tiles = {}
            t = xpool.tile([P, KO, C_CHUNK], bf16, tag="x")
            c0_0, cn_0 = seg_chunks[0][0]
            nc.scalar.dma_start(t[:, :, :cn_0], xT_r[:, :, c0_0:c0_0 + cn_0])
            x_tiles[(0, 0)] = t
            b1_sb = bpool.tile([P, NSLOT * FT], fp32)
            nc.scalar.dma_start(b1_sb[:], b1[:])
            b2_sb = bpool.tile([P, NSLOT * DT], fp32)
            nc.scalar.dma_start(b2_sb[:], b2[:])
            load_weights(0)
            cur_w1 = dict(w1_t)
            cur_w2 = dict(w2_t)
            for ci in range(1, len(seg_chunks[0])):
                x_tiles[(0, ci)] = load_x(0, ci)
            if NSLOT > 1:
                load_weights(1)
                nxt_w1, nxt_w2 = dict(w1_t), dict(w2_t)
                x_tiles[(1, 0)] = load_x(1, 0)

            for s in range(NSLOT):
                if s > 0:
                    cur_w1, cur_w2 = nxt_w1, nxt_w2
                    # queue the rest of this segment's x, then next segment's
                    # weights and first x chunk.
                    for ci in range(1, len(seg_chunks[s])):
                        x_tiles[(s, ci)] = load_x(s, ci)
                    if s + 1 < NSLOT:
                        load_weights(s + 1)
                        nxt_w1, nxt_w2 = dict(w1_t), dict(w2_t)
                        x_tiles[(s + 1, 0)] = load_x(s + 1, 0)

                for ci, (c0, cn) in enumerate(seg_chunks[s]):
                    x_sb = x_tiles.pop((s, ci))

                    h_sb = hpool.tile([P, FT, C_CHUNK], bf16, tag="h")
                    for ft in range(FT):
                        fb, fc = divmod(ft * P, F_BLK)
                        h_ps = hpsum.tile([P, PSUM_W], fp32, tag="hps")
                        for ko in range(KO):
                            nc.tensor.matmul(
                                h_ps[:, :cn],
                                cur_w1[(ko, fb)][:, fc:fc + P],
                                x_sb[:, ko, :cn],
                                start=(ko == 0),
                                stop=(ko == KO - 1),
                            )
                        # gelu(mm + b1) fused on ScalarE, cast to bf16 on write
                        nc.scalar.activation(
                            h_sb[:, ft, :cn], h_ps[:, :cn], AF.Gelu,
                            bias=b1_sb[:, s * FT + ft:s * FT + ft + 1],
                        )

                    last_chunk = (s == NSLOT - 1) and (ci == len(seg_chunks[s]) - 1)
                    for dt_ in range(DT):
                        y_ps = ypsum.tile([P, PSUM_W], fp32, tag="yps")
                        for fo in range(FT):
                            nc.tensor.matmul(
                                y_ps[:, :cn],
                                cur_w2[fo][:, dt_ * P:(dt_ + 1) * P],
                                h_sb[:, fo, :cn],
                                start=(fo == 0),
                                stop=(fo == FT - 1),
                            )
                        y_sb = ypool.tile([P, C_CHUNK], fp32, tag="y")
                        nc.vector.tensor_scalar_add(
                            y_sb[:, :cn], y_ps[:, :cn],
                            b2_sb[:, s * DT + dt_:s * DT + dt_ + 1],
                        )
                        # y-out stays on the sync queue: by the time a y DMA
                        # reaches the queue head its vector add has already
                        # run, and nothing urgent queues behind it. (On the
                        # Activation queue it would block the next chunk's
                        # gelu stream and stall the PE via PSUM backpressure.)
                        # The final chunk splits across both queues to halve
                        # the drain at kernel end (gelu is done by then).
                        y_eng = nc.scalar if (last_chunk and dt_ % 2) else nc.sync
                        y_eng.dma_start(
                            yT_r[:, dt_, offs[s] + c0:offs[s] + c0 + cn],
                            y_sb[:, :cn],
                        )

    return _split_drain_waits(nc)


def _to_bf16(a):
    """Fast float32 -> bfloat16 with round-to-nearest-even via bit ops."""
    a = np.ascontiguousarray(a, dtype=np.float32)
    u = a.view(np.uint32)
    r = ((u + 0x7FFF + ((u >> 16) & 1)) >> 16).astype(np.uint16)
    return r.view(ml_dtypes.bfloat16)


def kernel(hidden_states, Wg, bg, W1, b1, W2, b2):
    from concourse import bass_utils

    hs = np.ascontiguousarray(hidden_states, dtype=np.float32).reshape(B * S, D)

    # ---- Gate on host (float64): softmax over experts, top-2, renormalize
    logits = hs.astype(np.float64) @ np.asarray(Wg, np.float64).T
    logits += np.asarray(bg, np.float64)
    logits -= logits.max(axis=-1, keepdims=True)
    p = np.exp(logits)
    p /= p.sum(axis=-1, keepdims=True)

    i1 = p.argmax(axis=-1)
    rows = np.arange(B * S)
    p1 = p[rows, i1]
    pm = p.copy()
    pm[rows, i1] = -1.0
    i2 = pm.argmax(axis=-1)
    p2 = p[rows, i2]
    denom = p1 + p2
    g1 = (p1 / denom).astype(np.float32)
    g2 = (p2 / denom).astype(np.float32)

    # ---- Dispatch: token ids + combine weights per expert, split into two
    # token-halves; each half is served by two units (one per F-half).
    pieces = []  # (length, expert, ids, cw)
    for e in range(E):
        m1 = np.nonzero(i1 == e)[0]
        m2 = np.nonzero(i2 == e)[0]
        ids = np.concatenate([m1, m2])
        cw = np.concatenate([g1[m1], g2[m2]])
        # Even halves minimize total padded capacity (sum of group maxima).
        n0 = (len(ids) + 1) // 2
        for sl in (slice(0, n0), slice(n0, len(ids))):
            pieces.append((len(ids[sl]), e, ids[sl], cw[sl]))

    # 32 units = 16 token-pieces x 2 F-halves; sort by length, pack into
    # NSLOT groups of 8; slot length = group max; unit i of group g goes to
    # core i, slot g.
    units = []  # (length, piece_idx, fhalf)
    for pi, (ln, _e, _ids, _cw) in enumerate(pieces):
        for fh in range(2):
            units.append((ln, pi, fh))
    units.sort(key=lambda u: -u[0])
    assert len(units) == 8 * NSLOT
    groups = [units[8 * g:8 * (g + 1)] for g in range(NSLOT)]
    Ls = tuple(max(max(u[0] for u in grp), 1) for grp in groups)
    offs = [sum(Ls[:s]) for s in range(NSLOT)]
    T = sum(Ls)

    if Ls not in _compiled:
        _compiled[Ls] = _build_nc(Ls)
    nc = _compiled[Ls]

    W1b = [_to_bf16(np.asarray(W1[e], np.float32)) for e in range(E)]
    W2b = [_to_bf16(np.asarray(W2[e], np.float32)) for e in range(E)]

    in_maps = []
    placement = []  # per core: list over slots of (piece_idx, fhalf)
    for c in range(E):
        xT = np.zeros((D, T), dtype=ml_dtypes.bfloat16)
        w1s = np.zeros((NSLOT * D, F2), dtype=ml_dtypes.bfloat16)
        w2s = np.zeros((NSLOT * F2, D), dtype=ml_dtypes.bfloat16)
        b1s = np.zeros((P, NSLOT * FT), dtype=np.float32)
        b2s = np.zeros((P, NSLOT * DT), dtype=np.float32)
        slots = []
        for g in range(NSLOT):
            ln, pi, fh = groups[g][c]
            _ln, e, ids, _cw = pieces[pi]
            fsl = slice(fh * F2, (fh + 1) * F2)
            xT[:, offs[g]:offs[g] + ln] = _to_bf16(hs[ids]).T
            w1s[g * D:(g + 1) * D] = W1b[e][:, fsl]
            w2s[g * F2:(g + 1) * F2] = W2b[e][fsl]
            b1s[:, g * FT:(g + 1) * FT] = (
                np.asarray(b1[e], np.float32)[fsl].reshape(FT, P).T
            )
            if fh == 0:  # add b2 exactly once per token-piece
                b2s[:, g * DT:(g + 1) * DT] = (
                    np.asarray(b2[e], np.float32).reshape(DT, P).T
                )
            slots.append((pi, fh))
        placement.append(slots)
        in_maps.append({
            "xT": xT, "w1": w1s, "w2": w2s, "b1": b1s, "b2": b2s,
        })

    kwargs = {}
    if TRACE:
        import os as _os
        kwargs = dict(trace=True, trace_cores=list(range(E)))
        if _os.environ.get("MOE_TRACE_DIR"):
            _os.makedirs(_os.environ["MOE_TRACE_DIR"], exist_ok=True)
            kwargs["tmpdir"] = _os.environ["MOE_TRACE_DIR"]
    res = bass_utils.run_bass_kernel_spmd(nc, in_maps, list(range(E)), **kwargs)
    global LAST_RESULTS
    LAST_RESULTS = res

    out = np.zeros((B * S, D), dtype=np.float32)
    for c in range(E):
        yTc = res.results[c]["yT"]
        for g in range(NSLOT):
            pi, _fh = placement[c][g]
            ln, _e, ids, cw = pieces[pi]
            if ln:
                out[ids] += cw[:, None] * yTc[:, offs[g]:offs[g] + ln].T
    return out.reshape(B, S, D)


# revision 29
# speedup vs baseline: 1.1894x; 1.0153x over previous
"""Mixture-of-Experts (B=4, S=2048, D=1024, F=4096, E=8, top-2) on 8 trn2 NeuronCores.

Strategy: load-balanced expert parallelism. The tensor engine is the bottleneck
(bf16 roofline), so per-core work must be leveled. Each expert is split into
2 F-halves x 2 token-halves = 32 units of ~(C_e/2 tokens, F/2 cols). Units are
sorted by token count and packed into 8 cores x 4 fixed-length slots; slot
lengths (compile-time constants, shared by all cores) are the per-group maxima,
so per-core capacity ~= total/8 instead of the max-loaded expert.
  - Host: gate (softmax + top-2 + renorm) in float64, split + gather tokens per
    unit, transpose to [D, L] so the contraction dim lands on SBUF partitions.
  - Device (SPMD, identical program, per-core data): for each of 4 slots s:
    y^T = W2s^T @ gelu(W1s^T @ x^T + b1s) + b2s, weights streamed per slot
    (double buffered), tokens in chunks of <=512, PSUM accumulates in fp32.
    b2 is passed as zeros for F-half-1 units so it is added exactly once.
  - Host: combine with gate weights and scatter-add; the two F-half partials of
    a token-piece add up to the full FFN output.
"""

import copy
import sys

import numpy as np

for _p in ("/opt/trn_rl_repo", "/opt/pypackages"):
    if _p not in sys.path:
        sys.path.append(_p)

import ml_dtypes

B, S, D = 4, 2048, 1024
F = 4 * D
F2 = F // 2
E = 8
TOP_K = 2
P = 128
# PE cost model (HW-measured): a matmul takes N cycles at 2.4 GHz (N = moving
# free dim) + ~2.5 ns NX issue overhead, LDWEIGHTS hidden for N >= 128. So
# big chunks amortize overhead; 512 keeps one PSUM bank per tile.
C_CHUNK = 512
PSUM_W = 512
NSLOT = 4

KO = D // P           # 8  k-subtiles for the first matmul
FT = F2 // P          # 16 f-tiles (partition tiles of h) per F-half
DT = D // P           # 8  d-tiles (partition tiles of y)

# test-harness hooks (left off for grading)
TRACE = False
LAST_RESULTS = None

_compiled = {}


def _split_drain_waits(nc, max_waits=1):
    """This walrus build rejects instructions carrying more than one sync
    wait ("Too many sync wait commands"). Keep one wait on the instruction and
    move the excess onto NoOps inserted right before it on the same engine
    (engines are in-order, so blocking semantics are identical). Updates stay
    on the original instruction — moving them to a trailing NoOp could signal
    before the op's writes land."""
    import concourse.mybir as mybir

    m = nc.m
    new_module = copy.replace(m, functions=[])
    for function in m.functions:
        new_function = copy.replace(function, blocks=[])
        new_function.set_allocations_from_list(function.allocations)
        for block in function.blocks:
            out = []
            for inst in block.instructions:
                si = getattr(inst, "sync_info", None)
                on_wait = list(si.on_wait) if si is not None and si.on_wait else []
                if len(on_wait) > max_waits:
                    engine = getattr(inst, "engine", None)
                    extra, keep = on_wait[max_waits:], on_wait[:max_waits]
                    for j, w in enumerate(extra):
                        out.append(
                            mybir.InstNoOp(
                                name=f"{inst.name}-w{j}",
                                engine=engine,
                                sync_info=mybir.SyncInfo(on_wait=[w], on_update=[]),
                                bass_nofuse=True,
                            )
                        )
                    inst.sync_info = mybir.SyncInfo(
                        on_wait=keep,
                        on_update=list(si.on_update) if si.on_update else [],
                    )
                out.append(inst)
            new_function.blocks.append(copy.replace(block, instructions=out))
        new_module.functions.append(new_function)
    nc.m = new_module
    return nc


def _chunks_of(L):
    """Split L tokens into ceil(L/C_CHUNK) nearly-even chunks (each <= 512)."""
    if L <= 0:
        return []
    n = -(-L // C_CHUNK)
    base, rem = divmod(L, n)
    out, c0 = [], 0
    for i in range(n):
        cn = base + (1 if i < rem else 0)
        out.append((c0, cn))
        c0 += cn
    return out


def _build_nc(Ls):
    """Ls: tuple of NSLOT slot lengths (tokens per slot, same on every core)."""
    import concourse.bass as bass
    import concourse.mybir as mybir
    from concourse.tile import TileContext

    fp32 = mybir.dt.float32
    bf16 = mybir.dt.bfloat16
    AF = mybir.ActivationFunctionType

    T = sum(Ls)
    offs = [sum(Ls[:s]) for s in range(NSLOT)]

    nc = bass.Bass()
    xT = nc.declare_dram_parameter("xT", [D, T], bf16, isOutput=False)
    w1 = nc.declare_dram_parameter("w1", [NSLOT * D, F2], bf16, isOutput=False)
    w2 = nc.declare_dram_parameter("w2", [NSLOT * F2, D], bf16, isOutput=False)
    # biases come host-transposed to [128, n/128] so the DMA is contiguous
    b1 = nc.declare_dram_parameter("b1", [P, NSLOT * FT], fp32, isOutput=False)
    b2 = nc.declare_dram_parameter("b2", [P, NSLOT * DT], fp32, isOutput=False)
    yT = nc.declare_dram_parameter("yT", [D, T], fp32, isOutput=True)

    xT_r = xT.rearrange("(ko ki) t -> ki ko t", ki=P)          # [128, 8, T]
    w1_r = w1.rearrange("(g ki) f -> ki g f", ki=P)            # [128, 4*8, F2]
    w2_r = w2.rearrange("(g fi) d -> fi g d", fi=P)            # [128, 4*16, D]
    yT_r = yT.rearrange("(do di) t -> di do t", di=P)

    seg_chunks = [_chunks_of(L) for L in Ls]

    F_BLK = 512
    FB = F2 // F_BLK   # 4 w1 waves per slot

    with TileContext(nc) as tc:
        with (
            tc.tile_pool(name="wpool", bufs=2) as wpool,
            tc.tile_pool(name="bpool", bufs=1) as bpool,
            # bufs=5: every x DMA's buffer was last read >=1 segment ago, so
            # x DMAs carry no unresolved waits and never head-block the
            # in-order sync DMA queue (the v2 convoy that starved the PE).
            tc.tile_pool(name="xpool", bufs=5) as xpool,
            tc.tile_pool(name="hpool", bufs=1) as hpool,
            tc.tile_pool(name="ypool", bufs=8) as ypool,
            tc.tile_pool(name="hpsum", bufs=4, space="PSUM") as hpsum,
            tc.tile_pool(name="ypsum", bufs=4, space="PSUM") as ypsum,
        ):
            # Weight tiles stay flat [128, 512] / [128, 1024]: LDWEIGHTS from
            # slices of batched 3D tiles measures ~20 ns slower PER MATMUL
            # (+50us/kernel — it defeats the fast-weight-load path). The many
            # small DMAs' issue cost overlaps compute and is harmless.
            w1_t = {}
            w2_t = {}

            def load_weights(s):
                # f-block-major w1 waves so chunk-0's mm1 can start after
                # wave 0; w2 tiles follow (mm2 needs them one chunk later).
                for fb in range(FB):
                    for ko in range(KO):
                        t = wpool.tile([P, F_BLK], bf16, tag=f"w1_{ko}_{fb}")
                        nc.sync.dma_start(
                            t[:], w1_r[:, s * KO + ko, fb * F_BLK:(fb + 1) * F_BLK]
                        )
                        w1_t[(ko, fb)] = t
                for fo in range(FT):
                    t = wpool.tile([P, D], bf16, tag=f"w2_{fo}")
                    nc.sync.dma_start(t[:], w2_r[:, s * FT + fo, :])
                    w2_t[fo] = t

            def load_x(s, ci):
                c0, cn = seg_chunks[s][ci]
                t = xpool.tile([P, KO, C_CHUNK], bf16, tag="x")
                nc.sync.dma_start(
                    t[:, :, :cn], xT_r[:, :, offs[s] + c0:offs[s] + c0 + cn]
                )
                return t

            # prologue: first x chunk, biases, slot-0 weights, slot-0 x tail,
            # then slot-1 weights + first x so every DMA lands one segment
            # ahead of its consumer.
            # HAM warm-up: the PE clock gate sits at 1.2 GHz until ~3.4us of
            # sustained activity. The PE is idle during the startup DMAs
            # anyway, so burn that window on throwaway matmuls over a zeroed
            # scratch tile; real matmuls then start at 2.4 GHz.
            # Engine preamble ends ~7us and the first real matmul's data lands
            # ~13us; 24 N=256 throwaway matmuls fill exactly that window
            # (~3.4us of them cold, which is what flips the clock gate).
            warm_sb = bpool.tile([P, C_CHUNK], bf16)
            nc.vector.memset(warm_sb[:], 0)
            for _ in range(24):
                w_ps = hpsum.tile([P, PSUM_W], fp32, tag="hps")
                nc.tensor.matmul(
                    w_ps[:, :256], warm_sb[:, :P], warm_sb[:, :256],
                    start=True, stop=True,
                )

            # x00 + biases ride the Activation hwdge queue so the startup load
            # (first x chunk + first w1 block, ~2MB) runs on two queues in
            # parallel; gelu shares that queue but none of these carry waits.
            x_tiles = {}
            t = xpool.tile([P, KO, C_CHUNK], bf16, tag="x")
            c0_0, cn_0 = seg_chunks[0][0]
            nc.scalar.dma_start(t[:, :, :cn_0], xT_r[:, :, c0_0:c0_0 + cn_0])
            x_tiles[(0, 0)] = t
            b1_sb = bpool.tile([P, NSLOT * FT], fp32)
            nc.scalar.dma_start(b1_sb[:], b1[:])
            b2_sb = bpool.tile([P, NSLOT * DT], fp32)
            nc.scalar.dma_start(b2_sb[:], b2[:])
            load_weights(0)
            cur_w1 = dict(w1_t)
            cur_w2 = dict(w2_t)
            for ci in range(1, len(seg_chunks[0])):
                x_tiles[(0, ci)] = load_x(0, ci)
            if NSLOT > 1:
                load_weights(1)
                nxt_w1, nxt_w2 = dict(w1_t), dict(w2_t)
                x_tiles[(1, 0)] = load_x(1, 0)

            for s in range(NSLOT):
                if s > 0:
                    cur_w1, cur_w2 = nxt_w1, nxt_w2
                    # queue the rest of this segment's x, then next segment's
                    # weights and first x chunk.
                    for ci in range(1, len(seg_chunks[s])):
                        x_tiles[(s, ci)] = load_x(s, ci)
                    if s + 1 < NSLOT:
                        load_weights(s + 1)
                        nxt_w1, nxt_w2 = dict(w1_t), dict(w2_t)
                        x_tiles[(s + 1, 0)] = load_x(s + 1, 0)

                for ci, (c0, cn) in enumerate(seg_chunks[s]):
                    x_sb = x_tiles.pop((s, ci))

                    # h in two half-tiles: the tile framework tracks deps per
                    # tile, so with one h tile every chunk's first mm2 waits
                    # for ALL 16 gelus (~0.6-1us PE stall per chunk). Split,
                    # and mm2 starts once the first 8 gelus land.
                    h_lo = hpool.tile([P, FT // 2, C_CHUNK], bf16, tag="h_a")
                    h_hi = hpool.tile([P, FT // 2, C_CHUNK], bf16, tag="h_b")
                    h_half = [h_lo, h_hi]
                    for ft in range(FT):
                        fb, fc = divmod(ft * P, F_BLK)
                        h_ps = hpsum.tile([P, PSUM_W], fp32, tag="hps")
                        for ko in range(KO):
                            nc.tensor.matmul(
                                h_ps[:, :cn],
                                cur_w1[(ko, fb)][:, fc:fc + P],
                                x_sb[:, ko, :cn],
                                start=(ko == 0),
                                stop=(ko == KO - 1),
                            )
                        # gelu(mm + b1) fused on ScalarE, cast to bf16 on write
                        nc.scalar.activation(
                            h_half[ft // 8][:, ft % 8, :cn], h_ps[:, :cn], AF.Gelu,
                            bias=b1_sb[:, s * FT + ft:s * FT + ft + 1],
                        )

                    last_chunk = (s == NSLOT - 1) and (ci == len(seg_chunks[s]) - 1)
                    for dt_ in range(DT):
                        y_ps = ypsum.tile([P, PSUM_W], fp32, tag="yps")
                        for fo in range(FT):
                            nc.tensor.matmul(
                                y_ps[:, :cn],
                                cur_w2[fo][:, dt_ * P:(dt_ + 1) * P],
                                h_half[fo // 8][:, fo % 8, :cn],
                                start=(fo == 0),
                                stop=(fo == FT - 1),
                            )
                        y_sb = ypool.tile([P, C_CHUNK], fp32, tag="y")
                        nc.vector.tensor_scalar_add(
                            y_sb[:, :cn], y_ps[:, :cn],
                            b2_sb[:, s * DT + dt_:s * DT + dt_ + 1],
                        )
                        # y-out stays on the sync queue: by the time a y DMA
                        # reaches the queue head its vector add has already
                        # run, and nothing urgent queues behind it. (On the
                        # Activation queue it would block the next chunk's
                        # gelu stream and stall the PE via PSUM backpressure.)
                        # The final chunk splits across both queues to halve
                        # the drain at kernel end (gelu is done by then).
                        y_eng = nc.scalar if (last_chunk and dt_ % 2) else nc.sync
                        y_eng.dma_start(
                            yT_r[:, dt_, offs[s] + c0:offs[s] + c0 + cn],
                            y_sb[:, :cn],
                        )

    return _split_drain_waits(nc)


def _to_bf16(a):
    """Fast float32 -> bfloat16 with round-to-nearest-even via bit ops."""
    a = np.ascontiguousarray(a, dtype=np.float32)
    u = a.view(np.uint32)
    r = ((u + 0x7FFF + ((u >> 16) & 1)) >> 16).astype(np.uint16)
    return r.view(ml_dtypes.bfloat16)


def kernel(hidden_states, Wg, bg, W1, b1, W2, b2):
    from concourse import bass_utils

    hs = np.ascontiguousarray(hidden_states, dtype=np.float32).reshape(B * S, D)

    # ---- Gate on host (float64): softmax over experts, top-2, renormalize
    logits = hs.astype(np.float64) @ np.asarray(Wg, np.float64).T
    logits += np.asarray(bg, np.float64)
    logits -= logits.max(axis=-1, keepdims=True)
    p = np.exp(logits)
    p /= p.sum(axis=-1, keepdims=True)

    i1 = p.argmax(axis=-1)
    rows = np.arange(B * S)
    p1 = p[rows, i1]
    pm = p.copy()
    pm[rows, i1] = -1.0
    i2 = pm.argmax(axis=-1)
    p2 = p[rows, i2]
    denom = p1 + p2
    g1 = (p1 / denom).astype(np.float32)
    g2 = (p2 / denom).astype(np.float32)

    # ---- Dispatch: token ids + combine weights per expert, split into two
    # token-halves; each half is served by two units (one per F-half).
    pieces = []  # (length, expert, ids, cw)
    for e in range(E):
        m1 = np.nonzero(i1 == e)[0]
        m2 = np.nonzero(i2 == e)[0]
        ids = np.concatenate([m1, m2])
        cw = np.concatenate([g1[m1], g2[m2]])
        # Even halves minimize total padded capacity (sum of group maxima).
        n0 = (len(ids) + 1) // 2
        for sl in (slice(0, n0), slice(n0, len(ids))):
            pieces.append((len(ids[sl]), e, ids[sl], cw[sl]))

    # 32 units = 16 token-pieces x 2 F-halves; sort by length, pack into
    # NSLOT groups of 8; slot length = group max; unit i of group g goes to
    # core i, slot g.
    units = []  # (length, piece_idx, fhalf)
    for pi, (ln, _e, _ids, _cw) in enumerate(pieces):
        for fh in range(2):
            units.append((ln, pi, fh))
    units.sort(key=lambda u: -u[0])
    assert len(units) == 8 * NSLOT
    groups = [units[8 * g:8 * (g + 1)] for g in range(NSLOT)]
    Ls = tuple(max(max(u[0] for u in grp), 1) for grp in groups)
    offs = [sum(Ls[:s]) for s in range(NSLOT)]
    T = sum(Ls)

    if Ls not in _compiled:
        _compiled[Ls] = _build_nc(Ls)
    nc = _compiled[Ls]

    W1b = [_to_bf16(np.asarray(W1[e], np.float32)) for e in range(E)]
    W2b = [_to_bf16(np.asarray(W2[e], np.float32)) for e in range(E)]

    in_maps = []
    placement = []  # per core: list over slots of (piece_idx, fhalf)
    for c in range(E):
        xT = np.zeros((D, T), dtype=ml_dtypes.bfloat16)
        w1s = np.zeros((NSLOT * D, F2), dtype=ml_dtypes.bfloat16)
        w2s = np.zeros((NSLOT * F2, D), dtype=ml_dtypes.bfloat16)
        b1s = np.zeros((P, NSLOT * FT), dtype=np.float32)
        b2s = np.zeros((P, NSLOT * DT), dtype=np.float32)
        slots = []
        for g in range(NSLOT):
            ln, pi, fh = groups[g][c]
            _ln, e, ids, _cw = pieces[pi]
            fsl = slice(fh * F2, (fh + 1) * F2)
            xT[:, offs[g]:offs[g] + ln] = _to_bf16(hs[ids]).T
            w1s[g * D:(g + 1) * D] = W1b[e][:, fsl]
            w2s[g * F2:(g + 1) * F2] = W2b[e][fsl]
            b1s[:, g * FT:(g + 1) * FT] = (
                np.asarray(b1[e], np.float32)[fsl].reshape(FT, P).T
            )
            if fh == 0:  # add b2 exactly once per token-piece
                b2s[:, g * DT:(g + 1) * DT] = (
                    np.asarray(b2[e], np.float32).reshape(DT, P).T
                )
            slots.append((pi, fh))
        placement.append(slots)
        in_maps.append({
            "xT": xT, "w1": w1s, "w2": w2s, "b1": b1s, "b2": b2s,
        })

    kwargs = {}
    if TRACE:
        import os as _os
        kwargs = dict(trace=True, trace_cores=list(range(E)))
        if _os.environ.get("MOE_TRACE_DIR"):
            _os.makedirs(_os.environ["MOE_TRACE_DIR"], exist_ok=True)
            kwargs["tmpdir"] = _os.environ["MOE_TRACE_DIR"]
    res = bass_utils.run_bass_kernel_spmd(nc, in_maps, list(range(E)), **kwargs)
    global LAST_RESULTS
    LAST_RESULTS = res

    out = np.zeros((B * S, D), dtype=np.float32)
    for c in range(E):
        yTc = res.results[c]["yT"]
        for g in range(NSLOT):
            pi, _fh = placement[c][g]
            ln, _e, ids, cw = pieces[pi]
            if ln:
                out[ids] += cw[:, None] * yTc[:, offs[g]:offs[g] + ln].T
    return out.reshape(B, S, D)
